# revision 5
# baseline (speedup 1.0000x reference)
"""BCQ linear kernel for 8 TRN2 NeuronCores.

y = x @ dequant(qweight, alpha, beta)
  x: (4, 2048, 4096) f32, qweight: (128, 4, 4096) i32 bit-planes,
  alpha: (32, 4, 4096) f32, beta: (32, 4096) f32 -> y: (4, 2048, 4096) f32

Strategy: tensor-parallel over out_features (512 per core). Each core:
  - dequantizes its w slice (4096 x 512) on-chip into bf16 SBUF
      w[k, o] = sum_b alpha[g,b,o] * sign(bit_b(k,o)) + beta[g,o]
             = sum_b 2*alpha[g,b,o] * bit_b(k,o) + (beta[g,o] - sum_b alpha[g,b,o])
    bit extraction: host pre-shifts each packed word so that partition p's
    bit sits in the int32 sign position; on-chip it is a single
    (qs < 0) * (2*alpha) fused scalar_tensor_tensor op per k-tile.
  - streams x^T (bf16, host-prepared) and matmuls: out[m,o] tiles with
    lhsT = x^T[k,m] (stationary), rhs = w[k,o] (moving), f32 PSUM accum.
Host gathers the 8 out-feature slices.
"""
import sys

if "/opt/trn_rl_repo" not in sys.path:
    sys.path.insert(0, "/opt/trn_rl_repo")

import numpy as np
from ml_dtypes import bfloat16

import concourse.bacc as bacc
import concourse.bass as bass
import concourse.tile as tile
from concourse import mybir
from concourse.bass_utils import run_bass_kernel_spmd

IN_F = 4096
OUT_F = 4096
GROUP_SIZE = 128
WB = 4
BATCH = 4
SEQ = 2048
M_FULL = BATCH * SEQ          # 8192
N_CORES = 8
O_SH = OUT_F // N_CORES       # 512
P = 128

F32 = mybir.dt.float32
BF16 = mybir.dt.bfloat16
I32 = mybir.dt.int32
Alu = mybir.AluOpType


def build(M=M_FULL, K=IN_F, O=O_SH, debug=False):
    """Build the per-core Bass graph (SPMD: same graph, per-core inputs)."""
    assert M % 512 == 0 and K % P == 0
    KT = K // P                # k tiles (= quant groups, GROUP_SIZE == P)
    MC = M // 512              # m chunks of 512 rows (4 m-tiles each)
    P1C = min(2, MC)           # chunks processed k-outer during dequant

    nc = bacc.Bacc(None, target_bir_lowering=False, debug=debug)

    xt_d = nc.dram_tensor("xt", (K, M), BF16, kind="ExternalInput")
    qs_d = nc.dram_tensor("qs", (KT, P, WB, O), I32, kind="ExternalInput")
    al_d = nc.dram_tensor("al", (KT, WB, O), BF16, kind="ExternalInput")
    cc_d = nc.dram_tensor("cc", (KT, O), BF16, kind="ExternalInput")
    out_d = nc.dram_tensor("out", (M, O), F32, kind="ExternalOutput")

    xt_t = xt_d.rearrange("(kt p) m -> p kt m", p=P)   # (P, KT, M)

    with tile.TileContext(nc) as tc:
        with (
            tc.tile_pool(name="wpool", bufs=1) as wpool,
            tc.tile_pool(name="dq", bufs=3) as dq,
            tc.tile_pool(name="xs", bufs=3) as xs,
            tc.tile_pool(name="ys", bufs=4) as ys,
            tc.tile_pool(name="ps", bufs=8, space="PSUM") as ps,
        ):
            w_tiles = [
                wpool.tile([P, O], BF16, name=f"w{g}", tag=f"w{g}")
                for g in range(KT)
            ]

            # x chunks used in phase 1 (k-outer loop over first P1C chunks)
            x_chunks = {}
            for mc in range(P1C):
                xt_sb = xs.tile([P, KT, 512], BF16)
                nc.sync.dma_start(
                    out=xt_sb[:], in_=xt_t[:, :, mc * 512:(mc + 1) * 512]
                )
                x_chunks[mc] = xt_sb

            psum_p1 = [
                ps.tile([P, O], F32, name=f"ps{i}", tag="ps")
                for i in range(4 * P1C)
            ]

            # ---- phase 1: dequant k-tiles; matmul first P1C chunks k-outer ----
            for g in range(KT):
                qt = dq.tile([P, WB, O], I32, tag="qt")
                nc.sync.dma_start(out=qt[:], in_=qs_d[g, :, :, :])
                ab = dq.tile([P, WB, O], BF16, tag="ab")
                al_src = bass.AP(
                    tensor=al_d[:, :, :].tensor,
                    offset=g * WB * O,
                    ap=[[0, P], [O, WB], [1, O]],
                )
                nc.gpsimd.dma_start(out=ab[:], in_=al_src)
                cb = dq.tile([P, O], BF16, tag="cb")
                cc_src = bass.AP(
                    tensor=cc_d[:, :].tensor,
                    offset=g * O,
                    ap=[[0, P], [1, O]],
                )
                nc.gpsimd.dma_start(out=cb[:], in_=cc_src)

                # t[b] = (qs[b] < 0) * 2alpha[b]   (sign bit = the quant bit)
                t = dq.tile([P, WB, O], BF16, tag="t")
                nc.vector.scalar_tensor_tensor(
                    out=t[:], in0=qt[:], scalar=0.0, in1=ab[:],
                    op0=Alu.is_lt, op1=Alu.mult,
                )
                s = dq.tile([P, 2, O], BF16, tag="s")
                nc.vector.tensor_tensor(s[:], t[:, 0:2, :], t[:, 2:4, :], Alu.add)
                s2 = dq.tile([P, O], BF16, tag="s2")
                nc.vector.tensor_tensor(s2[:], s[:, 0, :], s[:, 1, :], Alu.add)
                nc.vector.tensor_tensor(w_tiles[g][:], s2[:], cb[:], Alu.add)

                # matmul this k-tile into the first P1C chunks' psums
                for mc in range(P1C):
                    for mt in range(4):
                        nc.tensor.matmul(
                            psum_p1[mc * 4 + mt][:],
                            x_chunks[mc][:, g, mt * 128:(mt + 1) * 128],
                            w_tiles[g][:],
                            start=(g == 0),
                            stop=(g == KT - 1),
                        )

            for mc in range(P1C):
                for mt in range(4):
                    y_sb = ys.tile([P, O], F32, tag="y")
                    nc.scalar.copy(y_sb[:], psum_p1[mc * 4 + mt][:])
                    row = (mc * 4 + mt) * 128
                    nc.sync.dma_start(out=out_d[row:row + 128, :], in_=y_sb[:])

            # ---- phase 2: remaining m chunks at full speed ----
            for mc in range(P1C, MC):
                xt_sb = xs.tile([P, KT, 512], BF16)
                nc.sync.dma_start(
                    out=xt_sb[:], in_=xt_t[:, :, mc * 512:(mc + 1) * 512]
                )
                for mt in range(4):
                    psum = ps.tile([P, O], F32, tag="ps")
                    for g in range(KT):
                        nc.tensor.matmul(
                            psum[:],
                            xt_sb[:, g, mt * 128:(mt + 1) * 128],
                            w_tiles[g][:],
                            start=(g == 0),
                            stop=(g == KT - 1),
                        )
                    y_sb = ys.tile([P, O], F32, tag="y")
                    nc.scalar.copy(y_sb[:], psum[:])
                    row = (mc * 4 + mt) * 128
                    nc.sync.dma_start(out=out_d[row:row + 128, :], in_=y_sb[:])

    return nc


def host_prep(x, qweight, alpha, beta, M=M_FULL, K=IN_F):
    """Full inputs -> per-core in_maps (shard over out_features)."""
    KT = K // P
    x2 = np.ascontiguousarray(x.reshape(M, K).astype(bfloat16).T)  # (K, M)

    k = np.arange(K)
    widx = (k // 32).astype(np.int64)
    shl = (31 - (k % 32)).astype(np.int32)

    o_sh = qweight.shape[-1] // N_CORES
    in_maps = []
    for c in range(N_CORES):
        sl = slice(c * o_sh, (c + 1) * o_sh)
        qw_s = qweight[:, :, sl]                       # (K/32, WB, o_sh) i32
        qs = (qw_s[widx] << shl[:, None, None]).astype(np.int32)
        qs = np.ascontiguousarray(qs.reshape(KT, P, WB, o_sh))
        al_s = alpha[:, :, sl].astype(np.float32)
        al2 = np.ascontiguousarray((2.0 * al_s).astype(bfloat16))
        ccb = np.ascontiguousarray(
            (beta[:, sl].astype(np.float32) - al_s.sum(axis=1)).astype(bfloat16)
        )
        in_maps.append({"xt": x2, "qs": qs, "al": al2, "cc": ccb})
    return in_maps


_NC_CACHE = {}


def _get_nc():
    if "nc" not in _NC_CACHE:
        nc = build()
        nc.compile()
        _NC_CACHE["nc"] = nc
    return _NC_CACHE["nc"]


def run(x, qweight, alpha, beta, trace=False, **kwargs):
    nc = _get_nc()
    in_maps = host_prep(x, qweight, alpha, beta)
    res = run_bass_kernel_spmd(
        nc, in_maps, core_ids=list(range(N_CORES)), trace=trace, **kwargs
    )
    y = np.concatenate(
        [np.asarray(res.results[c]["out"]) for c in range(N_CORES)], axis=1
    )
    y = np.ascontiguousarray(y.astype(np.float32)).reshape(BATCH, SEQ, OUT_F)
    return y, res


def kernel(x, qweight, alpha, beta):
    y, _ = run(
        np.asarray(x), np.asarray(qweight), np.asarray(alpha), np.asarray(beta)
    )
    return y
